# revision 9
# baseline (speedup 1.0000x reference)
"""GQA attention (B=2, S=2048, D=4096, 32 Q heads / 8 KV heads, head_dim=128,
RoPE, causal) on 8 Trainium2 NeuronCores, tensor-parallel over heads:
each core owns 4 Q heads + 1 KV head and a column shard of wq/wk/wv plus a
row shard of wo; the wo all-reduce is realized by summing the 8 partial
outputs on the host (the unshard/gather step).

Self-contained: all shapes hardcoded; only imports the system toolchain.
"""
import sys
import numpy as np

sys.path.insert(0, '/opt/trn_rl_repo')

import concourse.bass as bass          # noqa: E402
import concourse.mybir as mybir        # noqa: E402
import concourse.tile as tile          # noqa: E402
from concourse import bacc             # noqa: E402
from concourse import bass_utils       # noqa: E402

F32 = mybir.dt.float32
F32R = mybir.dt.float32r
AF = mybir.ActivationFunctionType

# ---- problem constants ----
N_HEADS = 32
N_KV_HEADS = 8
HEAD_DIM = 128
DIM = 4096
BATCH = 2
SEQ = 2048
N_CORES = 8
HQ = N_HEADS // N_CORES          # q heads per core = 4
SCALE = 1.0 / float(np.sqrt(HEAD_DIM))

_PROGRAM_CACHE = {}


def build_program(batch=BATCH, seq=SEQ, trace_label=""):
    """Emit the per-core Bass program (SPMD: identical on all 8 cores)."""
    P = 128
    TW = 512                      # token tile width
    DC = DIM // P                 # 32 contraction chunks
    TB = seq // TW                # t-tiles per batch
    SK = seq // P                 # sk chunks per batch

    nc = bacc.Bacc("TRN2", target_bir_lowering=False)

    xT = nc.dram_tensor("xT", [batch, DIM, seq], F32R, kind="ExternalInput").ap()
    wqT = nc.dram_tensor("wqT", [DIM, HQ * P], F32R, kind="ExternalInput").ap()
    wkT = nc.dram_tensor("wkT", [DIM, P], F32R, kind="ExternalInput").ap()
    wvT = nc.dram_tensor("wvT", [DIM, P], F32R, kind="ExternalInput").ap()
    woT = nc.dram_tensor("woT", [HQ * P, DIM], F32R, kind="ExternalInput").ap()
    cosT = nc.dram_tensor("cosT", [64, seq], F32, kind="ExternalInput").ap()
    sinT = nc.dram_tensor("sinT", [64, seq], F32, kind="ExternalInput").ap()
    masksI = nc.dram_tensor("masks", [4, P, TW], F32R, kind="ExternalInput").ap()
    identI = nc.dram_tensor("ident", [P, P], F32, kind="ExternalInput").ap()

    OUT = nc.dram_tensor("OUT", [DIM, batch * seq], F32, kind="ExternalOutput").ap()

    with tile.TileContext(nc) as tc:
        with (
            tc.tile_pool(name="glob", bufs=1) as glob,
            tc.tile_pool(name="dram", bufs=1, space="DRAM") as dpool,
        ):
            # ---- persistent SBUF state ----
            KT_sb = glob.tile([P, batch * seq], F32R)        # [d, tok]
            V_sb = glob.tile([P, batch * SK, P], F32R)       # [t, chunk, d]

            qt_dram = []
            for h in range(HQ):
                qtd = dpool.tile([P, batch * seq], F32R, tag=f"qtd{h}", name=f"qtd{h}")
                qt_dram.append(qtd)

            # ================= Phase 1: projections + RoPE =================
            with (
                tc.tile_pool(name="wts", bufs=1) as wts,
                tc.tile_pool(name="p1w", bufs=1) as p1w,
                tc.tile_pool(name="ps1", bufs=1, space="PSUM") as ps1,
            ):
                cos_sb = wts.tile([64, seq], F32)
                sin_sb = wts.tile([64, seq], F32)
                ident_sb = wts.tile([P, P], F32)
                nc.sync.dma_start(cos_sb[:], cosT[:])
                nc.sync.dma_start(sin_sb[:], sinT[:])
                nc.sync.dma_start(ident_sb[:], identI[:])
                wq_sb = wts.tile([P, DC, HQ * P], F32R)
                wk_sb = wts.tile([P, DC, P], F32R)
                wv_sb = wts.tile([P, DC, P], F32R)
                wqR = wqT.rearrange("(c p) m -> p c m", p=P)
                wkR = wkT.rearrange("(c p) m -> p c m", p=P)
                wvR = wvT.rearrange("(c p) m -> p c m", p=P)
                for c in range(DC):
                    nc.sync.dma_start(wq_sb[:, c, :], wqR[:, c, :])
                    nc.sync.dma_start(wk_sb[:, c, :], wkR[:, c, :])
                    nc.sync.dma_start(wv_sb[:, c, :], wvR[:, c, :])

                def rope_write(dst, ps, col, nm):
                    # drain the PSUM bank fast on the (idle) scalar engine,
                    # then do the rotation on DVE from SBUF at base 0
                    ct = cos_sb[:, col:col + TW]
                    st = sin_sb[:, col:col + TW]
                    qa = p1w.tile([64, TW], F32, tag="ra", bufs=3, name="qa")
                    qb = p1w.tile([64, TW], F32, tag="rb", bufs=3, name="qb")
                    nc.scalar.copy(qa[:], ps[0:64, :])
                    nc.scalar.copy(qb[:], ps[64:128, :])
                    t0 = p1w.tile([64, TW], F32, tag="rt0", bufs=2, name="t0")
                    t1 = p1w.tile([64, TW], F32, tag="rt1", bufs=2, name="t1")
                    t2 = p1w.tile([64, TW], F32, tag="rt2", bufs=2, name="t2")
                    t3 = p1w.tile([64, TW], F32, tag="rt3", bufs=2, name="t3")
                    nc.vector.tensor_mul(t0[:], qa[:], ct)
                    nc.vector.tensor_mul(t1[:], qb[:], st)
                    nc.vector.tensor_sub(dst[0:64, :], t0[:], t1[:])
                    nc.vector.tensor_mul(t2[:], qa[:], st)
                    nc.vector.tensor_mul(t3[:], qb[:], ct)
                    nc.vector.tensor_add(dst[64:128, :], t2[:], t3[:])

                for b in range(batch):
                    for tt in range(TB):
                        scol = tt * TW                 # within-batch col
                        gcol = b * seq + scol          # global col
                        q_ps = []
                        for h in range(HQ):
                            qp = ps1.tile([P, TW], F32, tag=f"q{h}", name=f"qps{h}")
                            q_ps.append(qp)
                        k_ps = ps1.tile([P, TW], F32, tag="k")
                        v_ps = ps1.tile([P, TW], F32, tag="v")
                        for c in range(DC):
                            xt = p1w.tile([P, TW], F32R, tag="xt", bufs=12)
                            nc.sync.dma_start(
                                xt[:], xT[b, c * P:(c + 1) * P, scol:scol + TW])
                            st = (c == 0)
                            sp = (c == DC - 1)
                            for h in range(HQ):
                                nc.tensor.matmul(
                                    q_ps[h][:], wq_sb[:, c, h * P:(h + 1) * P],
                                    xt[:], start=st, stop=sp)
                            nc.tensor.matmul(k_ps[:], wk_sb[:, c, :], xt[:],
                                             start=st, stop=sp)
                            nc.tensor.matmul(v_ps[:], wv_sb[:, c, :], xt[:],
                                             start=st, stop=sp)

                        # RoPE on Q (to DRAM scratch) and K (resident)
                        for h in range(HQ):
                            stq = p1w.tile([P, TW], F32R, tag="stq", bufs=3)
                            rope_write(stq, q_ps[h], scol, f"q{h}")
                            nc.sync.dma_start(
                                qt_dram[h][:, gcol:gcol + TW], stq[:])
                        rope_write(KT_sb[:, gcol:gcol + TW], k_ps, scol, "k")

                        # V: copy + per-128 transpose to natural layout
                        vtmp = p1w.tile([P, TW], F32, tag="vtmp", bufs=2)
                        nc.scalar.copy(vtmp[:], v_ps[:])
                        for q in range(TW // P):
                            tp = ps1.tile([P, P], F32, tag="vtp", bufs=2)
                            nc.tensor.transpose(
                                tp[:], vtmp[:, q * P:(q + 1) * P], ident_sb[:])
                            ci = (b * TB + tt) * (TW // P) + q
                            nc.scalar.copy(V_sb[:, ci, :], tp[:])

            # ================= Phase 2: attention =================
            with tc.tile_pool(name="p2o", bufs=1) as p2o:
              outT_sb = p2o.tile([P, HQ, batch * seq], F32R)
              masks_sb = p2o.tile([P, 4, TW], F32R)
              ones_sb = p2o.tile([P, 1], F32R)
              ones_f = p2o.tile([P, 1], F32)
              nc.sync.dma_start(masks_sb[:], masksI.rearrange("k p n -> p k n"))
              nc.any.memset(ones_f[:], 1.0)
              nc.vector.tensor_copy(ones_sb[:], ones_f[:])
              with (
                tc.tile_pool(name="p2w", bufs=1) as p2w,
                tc.tile_pool(name="ps2", bufs=1, space="PSUM") as ps2,
              ):
                for b in range(batch):
                    for h in range(HQ):
                        for j in range(TB):
                            gcol = b * seq + j * TW
                            qt = p2w.tile([P, TW], F32R, tag="qt", bufs=6)
                            nc.sync.dma_start(
                                qt[:], qt_dram[h][:, gcol:gcol + TW])
                            nsk = (j + 1) * (TW // P)
                            den_ps = ps2.tile([1, TW], F32, tag="den", bufs=2)
                            o_ps = ps2.tile([P, TW], F32, tag="opv", bufs=3)
                            for skc in range(nsk):
                                kcol = b * seq + skc * P
                                sc_ps = ps2.tile([P, TW], F32, tag="sc", bufs=3)
                                nc.tensor.matmul(
                                    sc_ps[:], KT_sb[:, kcol:kcol + P], qt[:],
                                    start=True, stop=True)
                                pt = p2w.tile([P, TW], F32R, tag="pt", bufs=6)
                                nc.scalar.activation(pt[:], sc_ps[:], AF.Exp,
                                                     scale=SCALE)
                                koff = skc - (TW // P) * j
                                if koff >= 0:
                                    nc.vector.tensor_mul(
                                        pt[:], pt[:], masks_sb[:, koff, :])
                                nc.tensor.matmul(den_ps[:], ones_sb[:], pt[:],
                                                 start=(skc == 0),
                                                 stop=(skc == nsk - 1))
                                ci = b * SK + skc
                                nc.tensor.matmul(o_ps[:], V_sb[:, ci, :], pt[:],
                                                 start=(skc == 0),
                                                 stop=(skc == nsk - 1))
                            rec = p2w.tile([1, TW], F32, tag="rc", bufs=2)
                            nc.vector.reciprocal(rec[:], den_ps[:])
                            bc = p2w.tile([P, TW], F32, tag="bc", bufs=2)
                            nc.gpsimd.partition_broadcast(bc[:], rec[:])
                            nc.vector.tensor_mul(
                                outT_sb[:, h, gcol:gcol + TW], o_ps[:], bc[:])

              # ================= Phase 3: output projection =================
              with (
                  tc.tile_pool(name="p3w", bufs=1) as p3w,
                  tc.tile_pool(name="ps3", bufs=1, space="PSUM") as ps3,
              ):
                  woR = woT.rearrange("(g p) m -> p g m", p=P)
                  for mg in range(DC // 4):
                      wom = p3w.tile([P, HQ, 4 * P], F32R, tag="wom", bufs=3)
                      nc.sync.dma_start(
                          wom[:], woR[:, :, mg * 4 * P:(mg + 1) * 4 * P])
                      for mi in range(4):
                          m = mg * 4 + mi
                          for t8 in range(batch * TB):
                              f_ps = ps3.tile([P, TW], F32, tag="f", bufs=4)
                              for h in range(HQ):
                                  nc.tensor.matmul(
                                      f_ps[:], wom[:, h, mi * P:(mi + 1) * P],
                                      outT_sb[:, h, t8 * TW:(t8 + 1) * TW],
                                      start=(h == 0), stop=(h == HQ - 1))
                              og = p3w.tile([P, TW], F32, tag="og", bufs=4)
                              nc.vector.tensor_copy(og[:], f_ps[:])
                              nc.sync.dma_start(
                                  OUT[m * P:(m + 1) * P, t8 * TW:(t8 + 1) * TW],
                                  og[:])

    nc.compile()
    return nc


_PERM = np.concatenate([np.arange(0, HEAD_DIM, 2), np.arange(1, HEAD_DIM, 2)])


def _make_masks(tw=512, p=128):
    masks = np.zeros((4, p, tw), np.float32)
    pp = np.arange(p)[:, None]
    ff = np.arange(tw)[None, :]
    for k in range(4):
        sub = ff // p
        within = ff % p
        masks[k] = np.where(sub > k, 1.0, np.where(sub == k, (pp <= within) * 1.0, 0.0))
    return masks


def prepare_core_inputs(x, freqs_cos, freqs_sin, wq, wk, wv, wo,
                        batch=BATCH, seq=SEQ):
    """Host-side shard + relayout (pure data movement, no arithmetic)."""
    xT = np.ascontiguousarray(np.asarray(x, np.float32).transpose(0, 2, 1))
    cosT = np.ascontiguousarray(np.asarray(freqs_cos, np.float32).T)
    sinT = np.ascontiguousarray(np.asarray(freqs_sin, np.float32).T)
    masks = _make_masks()
    ident = np.eye(128, dtype=np.float32)
    wq = np.asarray(wq, np.float32)
    wk = np.asarray(wk, np.float32)
    wv = np.asarray(wv, np.float32)
    wo = np.asarray(wo, np.float32)
    in_maps = []
    for c in range(N_CORES):
        wq_c = wq[c * HQ * HEAD_DIM:(c + 1) * HQ * HEAD_DIM]
        wq_c = wq_c.reshape(HQ, HEAD_DIM, DIM)[:, _PERM, :].reshape(HQ * HEAD_DIM, DIM)
        wk_c = wk[c * HEAD_DIM:(c + 1) * HEAD_DIM][_PERM, :]
        wv_c = wv[c * HEAD_DIM:(c + 1) * HEAD_DIM]
        wo_c = wo[:, c * HQ * HEAD_DIM:(c + 1) * HQ * HEAD_DIM]
        in_maps.append({
            "xT": xT,
            "wqT": np.ascontiguousarray(wq_c.T),
            "wkT": np.ascontiguousarray(wk_c.T),
            "wvT": np.ascontiguousarray(wv_c.T),
            "woT": np.ascontiguousarray(wo_c.T),
            "cosT": cosT,
            "sinT": sinT,
            "masks": masks,
            "ident": ident,
        })
    return in_maps


def run_sharded(in_maps, batch=BATCH, seq=SEQ, trace=False):
    key = (batch, seq)
    if key not in _PROGRAM_CACHE:
        _PROGRAM_CACHE[key] = build_program(batch, seq)
    nc = _PROGRAM_CACHE[key]
    res = bass_utils.run_bass_kernel_spmd(
        nc, in_maps, core_ids=list(range(len(in_maps))), trace=trace)
    return res


def kernel(x, freqs_cos, freqs_sin, wq, wk, wv, wo):
    b, s, _ = np.asarray(x, np.float32).shape
    in_maps = prepare_core_inputs(x, freqs_cos, freqs_sin, wq, wk, wv, wo,
                                  batch=b, seq=s)
    res = run_sharded(in_maps, batch=b, seq=s)
    acc = np.zeros((DIM, b * s), np.float64)
    for r in res.results:
        acc += np.asarray(r["OUT"], np.float64)
    out = acc.astype(np.float32).reshape(DIM, b, s).transpose(1, 2, 0)
    return np.ascontiguousarray(out)
